# revision 34
# baseline (speedup 1.0000x reference)
"""Multi-head attention (B=2, QL=KL=2048, D=1024, H=16) on 8 Trainium2 cores.

Sharding: data-parallel over batch (2) x tensor-parallel over heads (4 groups
of 4 heads) = 8 cores. Each core computes its batch's Q/K/V projections for
its 4 heads, causal+bias attention, and a partial Wo product; partials are
summed on the host (row-parallel reduction) and batches concatenated.

Head-PAIR dataflow per core (all matmuls bf16 in, f32 PSUM accumulate).
Heads (2m, 2m+1) share qh[m]/kh[m] tiles at partition halves 0-63 / 64-127:
  per i-block t, per pair m, per j-tile jt:
    S2[:, 0:512]    = khA.T @ qhA     (K=64, bank a)
    S2[:, 512:1024] = khB.T @ qhB     (K=64, bank a+1 -- fill/drain overlap)
    PT = exp(S2)                      (one ACT op covers both heads)
    PT *= rt                          (DVE bf16 2x; rt = host-side exp(bias),
                                       0 where masked -> exact masked softmax)
    aug_A += [vhA|1].T @ PT[:, 0:512]   (alternating aug banks)
    aug_B += [vhB|1].T @ PT[:, 512:1024]
  pair tail: aug[0:64] -> ot_un (gpsimd), aug[64] = l -> L[4,512] rows (ACT)
  lazy (pops next pair/block): 1/l = Exp(-Ln(L)) on ACT (batched, 4 lanes),
    gpsimd partition_broadcast, DVE mul -> ot, then Wo matmuls n-pair
    interleaved across PSUM banks, DVE evac, DMA out.

Perf notes vs the previous (inject-based) kernel, from NTFF traces:
  - NTFF MATMUL 'duration' is full latency (398+N)/2.4; back-to-back MMs to
    DIFFERENT PSUM banks overlap to ~N/2.4 issue rate, same-bank accumulation
    chains serialize at full latency. So every accumulation chain here
    (projection kt-loops, Wo m-loops, AV j-loops) is interleaved across two
    PSUM tiles/banks.
  - The 160 fp8 identity-inject bias matmuls (34us of PE stream) are gone:
    bias is applied as a DVE multiply (47us DVE, which has slack) with the
    exp folded in on the host.
  - nc.vector.reciprocal on [1,512] is ~3.3us (single-lane, 8 cyc/elem);
    1/l is instead computed as exp(-ln l) on ACT over [4,512] (two 570ns
    ops per block), deferred into the next block so the strict ACT FIFO
    never blocks the exp stream.
Masking is folded into the bias tiles on the host (exact 0 where masked).
Softmax uses no max-subtraction: scores are ~N(0,1) by construction.
"""

import math

import numpy as np
import ml_dtypes

import concourse.bass as bass
from concourse import bacc
import concourse.mybir as mybir
import concourse.tile as tile
from concourse.bass_utils import run_bass_kernel_spmd

dt = mybir.dt
bf16 = ml_dtypes.bfloat16

B, QL, KL, D, H, DH = 2, 2048, 2048, 1024, 16, 64
N_CORES = 8
HPC = 4            # heads per core
GROUPS = N_CORES // B  # 4 head-groups
IB = 512           # i-block width (softmax rows per block)
JT = 128           # j-tile height
N_IB = QL // IB
N_JT = KL // JT
KT = D // 128      # contraction tiles for projections


def classify_tiles(attn_mask, key_padding_mask):
    """Per i-block list of j-tiles that have at least one valid entry for at
    least one batch (uniform across cores; fully-masked tiles are skipped)."""
    m = np.asarray(attn_mask, dtype=bool)
    kp = np.asarray(key_padding_mask, dtype=bool)
    kp_any = kp.any(axis=0)  # [KL] valid for some batch
    classes = []
    for t in range(N_IB):
        mi = m[t * IB : (t + 1) * IB]
        row = []
        for jt in range(N_JT):
            v = mi[:, jt * JT : (jt + 1) * JT] & kp_any[jt * JT : (jt + 1) * JT][None, :]
            if v.any():
                row.append(jt)
        classes.append(row)
    return classes


def build_nc(classes, repeats=1, skip=()):
    skip = set(skip)
    """Build the SPMD Bass program.

    skip flags (ablation):
      dverecip-- vector.reciprocal on [1,512] instead of ACT exp(-ln l)
      woact   -- Wo PSUM evac on ACT instead of DVE
    """
    # rpb units are [128, 1024] (headA|headB) per (t, m, jt), DMA'd two
    # j-tiles at a time as [128, 2048]
    n_units = sum(2 * len(row) for row in classes)
    n_pair = n_units // 2

    nc = bacc.Bacc("TRN2", target_bir_lowering=False, debug=False)
    qT = nc.dram_tensor("qT", [D, QL], dt.bfloat16, kind="ExternalInput")
    kTd = nc.dram_tensor("kT", [D, KL], dt.bfloat16, kind="ExternalInput")
    vTd = nc.dram_tensor("vT", [D, KL], dt.bfloat16, kind="ExternalInput")
    wqT = nc.dram_tensor("wqT", [D, HPC * DH], dt.bfloat16, kind="ExternalInput")
    wkT = nc.dram_tensor("wkT", [D, HPC * DH], dt.bfloat16, kind="ExternalInput")
    wvT = nc.dram_tensor("wvT", [D, HPC * DH], dt.bfloat16, kind="ExternalInput")
    woT = nc.dram_tensor("woT", [HPC * DH, D], dt.bfloat16, kind="ExternalInput")
    rpbM = nc.dram_tensor("rpbM", [max(n_pair, 1), JT, 4 * IB],
                          dt.bfloat16, kind="ExternalInput")
    outP = nc.dram_tensor("outP", [D, QL], dt.bfloat16, kind="ExternalOutput")

    Exp = mybir.ActivationFunctionType.Exp
    Ln = mybir.ActivationFunctionType.Ln
    Copy = mybir.ActivationFunctionType.Copy

    with tile.TileContext(nc) as tc:
        with (
            tc.tile_pool(name="wp", bufs=1) as wp,
            tc.tile_pool(name="persist", bufs=1) as pers,
            tc.tile_pool(name="xq", bufs=6) as xq,
            tc.tile_pool(name="ptp", bufs=3) as ptp,
            tc.tile_pool(name="rpbp", bufs=6) as rpbp,
            tc.tile_pool(name="smallp", bufs=2) as smallp,
            tc.tile_pool(name="rbp", bufs=3) as rbp,
            tc.tile_pool(name="osb", bufs=3) as osbp,
            tc.tile_pool(name="psA", bufs=2, space="PSUM") as psA,
            tc.tile_pool(name="psS", bufs=2, space="PSUM") as psS,
            tc.tile_pool(name="psG", bufs=2, space="PSUM") as psG,
        ):

            def body():
                wq_t = wp.tile([128, KT, 256], dt.bfloat16, tag="wq")
                wk_t = wp.tile([128, KT, 256], dt.bfloat16, tag="wk")
                wv_t = wp.tile([128, KT, 256], dt.bfloat16, tag="wv")
                wo_t = wp.tile([128, 2, 1024], dt.bfloat16, tag="wo")

                # chunked persistent activation tiles (fine-grained deps so
                # early attention blocks can start before projections finish)
                qh = [[pers.tile([128, 512], dt.bfloat16, name=f"qh{m}_{c}", tag=f"qh{m}_{c}")
                       for c in range(QL // 512)] for m in range(2)]
                kh = [[pers.tile([128, 512], dt.bfloat16, name=f"kh{m}_{c}", tag=f"kh{m}_{c}")
                       for c in range(KL // 512)] for m in range(2)]
                vh = [pers.tile([128, HPC, 68], dt.bfloat16, name=f"vh{t}", tag=f"vh{t}")
                      for t in range(N_JT)]
                # unnormalized out^T and normalized (bf16) out^T per i-block
                otu = [pers.tile([128, 2, 512], dt.bfloat16, name=f"otu{t}", tag=f"otu{t}")
                       for t in range(N_IB)]
                ot = [pers.tile([128, 2, 512], dt.bfloat16, name=f"ot{t}", tag=f"ot{t}")
                      for t in range(N_IB)]
                # vh ones columns: memset upfront, off any critical path
                for t in range(N_JT):
                    nc.gpsimd.memset(vh[t][:, :, 64:65], 1.0)

                rpb_pre = {}

                def rpb_tile():
                    return rpbp.tile([JT, 2048], dt.bfloat16, tag="rpb", name="rt")

                def prefetch_rpb():
                    for i in range(min(len(classes[0]), 4)):
                        rt = rpb_tile()
                        nc.sync.dma_start(out=rt[:], in_=rpbM[i])
                        rpb_pre[i] = rt

                pending = []  # projection units: popped between j-tiles, and
                              # force-drained at block start
                lazy_rdy = []   # deferred norm/Wo units, poppable now
                lazy_new = []   # emitted this pair-loop; promoted at next pair

                xts = {}  # (kind, c) -> in-flight xt tile

                def dma_trio(c):
                    for src, kind in ((qT, "q"), (kTd, "k"), (vTd, "v")):
                        # one DMA per chunk: each issue costs ~630ns of
                        # serial SP time, which dwarfs fine-slicing benefits
                        xt = xq.tile([128, KT, 512], dt.bfloat16, tag="x", name="xt")
                        nc.sync.dma_start(
                            out=xt[:],
                            in_=src.ap()[:, c * 512 : (c + 1) * 512].rearrange(
                                "(k p) t -> p k t", p=128
                            ),
                        )
                        xts[(kind, c)] = xt

                def enqueue_trio(c):
                    for w_t, kind in ((wq_t, "q"), (wk_t, "k"), (wv_t, "v")):
                        enqueue_one(w_t, kind, c)

                def enqueue_one(w_t, kind, c):
                    xt = xts.pop((kind, c))
                    if kind in ("q", "k"):
                        dst = qh if kind == "q" else kh

                        # kt-chain interleaved across both m outputs (two PSUM
                        # banks) so consecutive MMs overlap fill/drain
                        def qk_unit(k0, k1, pps, xt=xt, w_t=w_t, dst=dst, c=c):
                            for kt in range(k0, k1):
                                for m in range(2):
                                    nc.tensor.matmul(
                                        pps[m][:],
                                        w_t[:, kt, m * 128 : (m + 1) * 128],
                                        xt[:, kt, :],
                                        start=(kt == 0),
                                        stop=(kt == KT - 1),
                                    )
                            if k1 == KT:
                                for m in range(2):
                                    nc.vector.tensor_copy(dst[m][c][:], pps[m][:])

                        pps = [psA.tile([128, 512], dt.float32, tag="mm", name=f"pp{m}")
                               for m in range(2)]
                        for k0, k1 in ((0, KT // 2), (KT // 2, KT)):
                            pending.append(lambda k0=k0, k1=k1, pps=pps, f=qk_unit: f(k0, k1, pps))
                    else:

                        # tsub pairs interleaved across two PSUM banks
                        def v_unit(t0, t1, pvs, xt=xt, c=c):
                            for kt in range(KT):
                                for i, ts in enumerate((t0, t1)):
                                    nc.tensor.matmul(
                                        pvs[i][:],
                                        xt[:, kt, ts * 128 : (ts + 1) * 128],
                                        wv_t[:, kt, :],
                                        start=(kt == 0),
                                        stop=(kt == KT - 1),
                                    )
                            for i, ts in enumerate((t0, t1)):
                                t = c * 4 + ts
                                nc.scalar.activation(
                                    vh[t][:, :, 0:64],
                                    pvs[i][:].rearrange("p (h c) -> p h c", h=HPC),
                                    Copy,
                                )

                        for t0, t1 in ((0, 1), (2, 3)):
                            pvs = [psA.tile([128, 256], dt.float32, tag="mm", name=f"pv{i}")
                                   for i in range(2)]
                            pending.append(lambda t0=t0, t1=t1, pvs=pvs, f=v_unit: f(t0, t1, pvs))

                def wo_unit(t, n0):
                    # two n-tiles interleaved across two PSUM banks
                    pws = [psA.tile([128, 512], dt.float32, tag="mm", name=f"pw{i}")
                           for i in range(2)]
                    for m in range(2):
                        for i in range(2):
                            nc.tensor.matmul(
                                pws[i][:],
                                wo_t[:, m, (n0 + i) * 128 : (n0 + i + 1) * 128],
                                ot[t][:, m, :],
                                start=(m == 0),
                                stop=(m == 1),
                            )
                    for i in range(2):
                        n = n0 + i
                        ob = osbp.tile([128, 512], dt.bfloat16, tag="ob")
                        if "woact" in skip:
                            nc.scalar.activation(ob[:], pws[i][:], Copy)
                        else:
                            nc.vector.tensor_copy(ob[:], pws[i][:])
                        if t == N_IB - 1:
                            # tail-critical: halve per-queue transfer latency
                            for hh in range(2):
                                nc.sync.dma_start(
                                    out=outP[n * 128 : (n + 1) * 128,
                                             t * IB + hh * 256 : t * IB + (hh + 1) * 256],
                                    in_=ob[:, hh * 256 : (hh + 1) * 256],
                                )
                        else:
                            nc.sync.dma_start(
                                out=outP[n * 128 : (n + 1) * 128, t * IB : (t + 1) * IB],
                                in_=ob[:],
                            )

                def pop_pending(allow_lazy=True):
                    if pending:
                        pending.pop(0)()
                    elif lazy_rdy and allow_lazy:
                        lazy_rdy.pop(0)()

                # ---- interleaved: attention i-block t runs while chunk t+1 of
                # the projections streams in between its j-tiles (causal: block
                # t only reads k/v chunks <= t) ----
                rpb_i = 0  # DMA'd rpb pair counter
                # block 0's inputs are emitted eagerly, each weight landing
                # just before the x-chunk that needs it; chunk 1's x-DMAs
                # are issued in the preamble too so block 0 (short) can
                # hide chunk 1's projections without stalling on transfers
                nc.sync.dma_start(out=wq_t[:], in_=wqT.ap().rearrange("(k p) c -> p k c", p=128))
                for src, kind in ((qT, "q"),):
                    xt = xq.tile([128, KT, 512], dt.bfloat16, tag="x", name="xt")
                    nc.sync.dma_start(out=xt[:], in_=src.ap()[:, 0:512].rearrange("(k p) t -> p k t", p=128))
                    xts[(kind, 0)] = xt
                enqueue_one(wq_t, "q", 0)
                while pending:
                    pop_pending()
                nc.sync.dma_start(out=wk_t[:], in_=wkT.ap().rearrange("(k p) c -> p k c", p=128))
                for src, kind in ((kTd, "k"), (vTd, "v")):
                    xt = xq.tile([128, KT, 512], dt.bfloat16, tag="x", name="xt")
                    nc.sync.dma_start(out=xt[:], in_=src.ap()[:, 0:512].rearrange("(k p) t -> p k t", p=128))
                    xts[(kind, 0)] = xt
                enqueue_one(wk_t, "k", 0)
                while pending:
                    pop_pending()
                prefetch_rpb()
                nc.sync.dma_start(out=wv_t[:], in_=wvT.ap().rearrange("(k p) c -> p k c", p=128))
                enqueue_one(wv_t, "v", 0)
                while pending:
                    pop_pending()
                nc.sync.dma_start(out=wo_t[:], in_=woT.ap().rearrange("(k p) c -> p k c", p=128))
                dma_trio(1)

                for t in range(N_IB):
                    row = classes[t]
                    n_row = len(row)
                    while pending:  # anything block t needs must be emitted now
                        pop_pending()
                    if t + 1 < N_IB:
                        enqueue_trio(t + 1)
                    if t + 2 < N_IB:
                        dma_trio(t + 2)
                    for m in range(2):
                        # promote deferred units from the previous pair-loop
                        lazy_rdy.extend(lazy_new)
                        lazy_new.clear()
                        # per-head ln(l) rows; base-0 [1,512] f32 tiles (the
                        # layout gpsimd partition_broadcast is known-good for)
                        Lt = [smallp.tile([1, 512], dt.float32, tag="L1", name="Lh")
                              for _ in range(2)]
                        augs = [psG.tile([65, 512], dt.float32, tag="aug", name=f"aug{i}")
                                for i in range(2)]
                        pends = []  # software-pipeline: AV(jt) issues after QK/exp(jt+1)

                        def av_pair(jj, jt, PT, m=m, augs=augs):
                            for i in range(2):
                                nc.tensor.matmul(
                                    augs[i][:],
                                    vh[jt][:, 2 * m + i, 0:65],
                                    PT[:, i * 512 : (i + 1) * 512],
                                    start=(jj == 0),
                                    stop=(jj == n_row - 1),
                                )

                        half = None
                        for jj, jt in enumerate(row):
                            if jj % 2 == 0:
                                # rpb pairs: two j-tiles per DMA
                                if rpb_i in rpb_pre:
                                    rt2 = rpb_pre.pop(rpb_i)
                                else:
                                    rt2 = rpb_tile()
                                    nc.sync.dma_start(out=rt2[:], in_=rpbM[rpb_i])
                                rpb_i += 1
                                half = 0
                            else:
                                half = 1
                            S2 = psS.tile([128, 1024], dt.float32, tag="s2")
                            PT = ptp.tile([128, 1024], dt.bfloat16, tag="pt")
                            for i in range(2):
                                hp = 64 * i
                                nc.tensor.matmul(
                                    S2[:, i * 512 : (i + 1) * 512],
                                    kh[m][jt // 4][hp : hp + 64, (jt % 4) * 128 : (jt % 4 + 1) * 128],
                                    qh[m][t][hp : hp + 64, :],
                                    start=True, stop=True,
                                )
                            nc.scalar.activation(PT[:], S2[:], Exp)
                            nc.vector.tensor_mul(
                                PT[:], PT[:], rt2[:, half * 1024 : (half + 1) * 1024]
                            )
                            pends.append((jj, jt, PT))
                            if len(pends) > 2:
                                av_pair(*pends.pop(0))
                            pop_pending()
                        while pends:
                            av_pair(*pends.pop(0))
                        # pair tail: evacuate augs + l rows, split evenly
                        # across ACT/DVE so neither FIFO blocks the next
                        # pair's exp/mul stream for long (gpsimd cannot
                        # read PSUM)
                        for i in range(2):
                            hp = 64 * i
                            if i == 0:
                                nc.scalar.activation(
                                    otu[t][hp : hp + 64, m, :], augs[i][0:64, :], Copy
                                )
                                nc.scalar.activation(Lt[i][:], augs[i][64:65, :], Copy)
                            else:
                                nc.vector.tensor_copy(
                                    otu[t][hp : hp + 64, m, :], augs[i][0:64, :]
                                )
                                nc.vector.tensor_copy(Lt[i][:], augs[i][64:65, :])

                        # deferred normalization for this pair (pops later, so
                        # the ACT FIFO isn't blocked behind the AV chain)
                        def norm_unit(t=t, m=m, Lt=Lt):
                            # 1/l without ACT table switches (Ln thrashes the
                            # exp table set, ~2.6us per swap) and without the
                            # single-lane [1,512] reciprocal (8 cyc/elem =
                            # 3.3us): DMA-spread l across 128 partitions,
                            # 170ns DVE reciprocal, DMA back, gpsimd
                            # partition-broadcast. All compute-dependent
                            # steps sit on the idle Pool/DVE queues so the
                            # PE/SP streams never wait on this chain.
                            for i in range(2):
                                hp = 64 * i
                                ls = smallp.tile([128, 4], dt.float32, tag="ls", name="ls")
                                nc.gpsimd.dma_start(out=ls[:], in_=Lt[i][:])
                                rs = smallp.tile([128, 4], dt.float32, tag="rs", name="rs")
                                nc.vector.reciprocal(rs[:], ls[:])
                                rc = smallp.tile([1, 512], dt.float32, tag="L1r", name="Lr")
                                nc.gpsimd.dma_start(out=rc[:], in_=rs[:])
                                rb = rbp.tile([128, 512], dt.float32, tag="rb", name="rb")
                                nc.gpsimd.partition_broadcast(rb[:], rc[:])
                                nc.vector.tensor_mul(
                                    ot[t][hp : hp + 64, m, :],
                                    otu[t][hp : hp + 64, m, :],
                                    rb[hp : hp + 64, :],
                                )

                        lazy_new.append(norm_unit)
                    # Wo partials for this i-block (needs both pairs' ot): pop
                    # during block t+1 instead of stalling the PE behind the
                    # normalization chain
                    for n0 in range(0, 8, 2):
                        lazy_new.append(lambda n0=n0, t=t: wo_unit(t, n0))
                while pending or lazy_rdy or lazy_new:
                    lazy_rdy.extend(lazy_new)
                    lazy_new.clear()
                    while pending or lazy_rdy:
                        pop_pending()

            if repeats == 1:
                body()
            else:
                hint = (mybir.EngineType.PE, mybir.EngineType.Activation,
                        mybir.EngineType.DVE, mybir.EngineType.SP,
                        mybir.EngineType.Pool)
                with tc.For_i(0, repeats, 1, hint_engines=hint):
                    body()

    nc.finalize()
    return nc


def make_in_maps(q, k, v, attn_mask, key_padding_mask, rel_pos_bias, Wq, Wk, Wv, Wo, classes, skip=()):
    q = np.asarray(q, np.float32)
    k = np.asarray(k, np.float32)
    v = np.asarray(v, np.float32)
    Wq = np.asarray(Wq, np.float32)
    Wk = np.asarray(Wk, np.float32)
    Wv = np.asarray(Wv, np.float32)
    Wo = np.asarray(Wo, np.float32)
    rpb = np.asarray(rel_pos_bias, np.float32)
    am = np.asarray(attn_mask, bool)
    kp = np.asarray(key_padding_mask, bool)

    scale = np.float32(1.0 / math.sqrt(DH))
    n_units = sum(2 * len(row) for row in classes)
    n_pair = n_units // 2

    in_maps = []
    for core in range(N_CORES):
        b = core // GROUPS
        g = core % GROUPS
        h0 = g * HPC
        r0 = h0 * DH

        qTc = q[b].T.astype(bf16)
        kTc = k[b].T.astype(bf16)
        vTc = v[b].T.astype(bf16)
        wqTc = ((Wq[r0 : r0 + HPC * DH] * scale).T).astype(bf16)
        wkTc = Wk[r0 : r0 + HPC * DH].T.astype(bf16)
        wvTc = Wv[r0 : r0 + HPC * DH].T.astype(bf16)
        woTc = np.ascontiguousarray(Wo[:, r0 : r0 + HPC * DH].T).astype(bf16)

        # bias tiles: exp(rel_pos_bias)^T where valid, exactly 0 where masked.
        # Unit = [128, 1024] (headA|headB) per (t, m, jt); packed two j-tiles
        # per DMA row in device iteration order.
        validT = (am & kp[b][None, :]).T  # [KL, QL]
        rpbm_arr = np.zeros((max(n_pair, 1), JT, 2048), dtype=bf16)
        iu = 0
        for t in range(N_IB):
            ts = slice(t * IB, (t + 1) * IB)
            for m in range(2):
                row = classes[t]
                for jj, jt in enumerate(row):
                    js = slice(jt * JT, (jt + 1) * JT)
                    vT = validT[js, ts]
                    pr, half = divmod(iu, 2)
                    for i in range(2):
                        rT = rpb[h0 + 2 * m + i].T  # [KL, QL] view
                        tilev = np.where(vT, np.exp(rT[js, ts]), 0.0).astype(bf16)
                        off = half * 1024 + i * 512
                        rpbm_arr[pr, :, off : off + 512] = tilev
                    iu += 1
        assert iu == n_units

        in_maps.append(
            {
                "qT": qTc, "kT": kTc, "vT": vTc,
                "wqT": wqTc, "wkT": wkTc, "wvT": wvTc, "woT": woTc,
                "rpbM": rpbm_arr,
            }
        )
    return in_maps


_CACHE = {}


def _get_nc(classes, repeats=1, skip=()):
    key = (tuple(tuple(row) for row in classes), repeats, tuple(sorted(skip)))
    if key not in _CACHE:
        _CACHE[key] = build_nc(classes, repeats, skip)
    return _CACHE[key]


def kernel(q, k, v, attn_mask, key_padding_mask, rel_pos_bias, Wq, Wk, Wv, Wo):
    classes = classify_tiles(attn_mask, key_padding_mask)
    nc = _get_nc(classes)
    in_maps = make_in_maps(
        q, k, v, attn_mask, key_padding_mask, rel_pos_bias, Wq, Wk, Wv, Wo, classes
    )
    res = run_bass_kernel_spmd(nc, in_maps, list(range(N_CORES))).results
    out = np.zeros((B, QL, D), np.float32)
    for core in range(N_CORES):
        out[core // GROUPS] += res[core]["outP"].T
    return out


# revision 37
# speedup vs baseline: 1.0475x; 1.0475x over previous
"""Multi-head attention (B=2, QL=KL=2048, D=1024, H=16) on 8 Trainium2 cores.

Sharding: data-parallel over batch (2) x tensor-parallel over heads (4 groups
of 4 heads) = 8 cores. Each core computes its batch's Q/K/V projections for
its 4 heads, causal+bias attention, and a partial Wo product; partials are
summed on the host (row-parallel reduction) and batches concatenated.

Head-PAIR dataflow per core (all matmuls bf16 in, f32 PSUM accumulate).
Heads (2m, 2m+1) share qh[m]/kh[m] tiles at partition halves 0-63 / 64-127:
  per i-block t, per pair m, per j-tile jt:
    S2[:, 0:512]    = khA.T @ qhA     (K=64, bank a)
    S2[:, 512:1024] = khB.T @ qhB     (K=64, bank a+1 -- fill/drain overlap)
    PT = exp(S2)                      (one ACT op covers both heads)
    PT *= rt                          (DVE bf16 2x; rt = host-side exp(bias),
                                       0 where masked -> exact masked softmax)
    aug_A += [vhA|1].T @ PT[:, 0:512]   (alternating aug banks)
    aug_B += [vhB|1].T @ PT[:, 512:1024]
  pair tail: aug[0:64] -> ot_un (gpsimd), aug[64] = l -> L[4,512] rows (ACT)
  lazy (pops next pair/block): 1/l = Exp(-Ln(L)) on ACT (batched, 4 lanes),
    gpsimd partition_broadcast, DVE mul -> ot, then Wo matmuls n-pair
    interleaved across PSUM banks, DVE evac, DMA out.

Perf notes vs the previous (inject-based) kernel, from NTFF traces:
  - NTFF MATMUL 'duration' is full latency (398+N)/2.4; back-to-back MMs to
    DIFFERENT PSUM banks overlap to ~N/2.4 issue rate, same-bank accumulation
    chains serialize at full latency. So every accumulation chain here
    (projection kt-loops, Wo m-loops, AV j-loops) is interleaved across two
    PSUM tiles/banks.
  - The 160 fp8 identity-inject bias matmuls (34us of PE stream) are gone:
    bias is applied as a DVE multiply (47us DVE, which has slack) with the
    exp folded in on the host.
  - nc.vector.reciprocal on [1,512] is ~3.3us (single-lane, 8 cyc/elem);
    1/l is instead computed as exp(-ln l) on ACT over [4,512] (two 570ns
    ops per block), deferred into the next block so the strict ACT FIFO
    never blocks the exp stream.
Masking is folded into the bias tiles on the host (exact 0 where masked).
Softmax uses no max-subtraction: scores are ~N(0,1) by construction.
"""

import math

import numpy as np
import ml_dtypes

import concourse.bass as bass
from concourse import bacc
import concourse.mybir as mybir
import concourse.tile as tile
from concourse.bass_utils import run_bass_kernel_spmd

dt = mybir.dt
bf16 = ml_dtypes.bfloat16

B, QL, KL, D, H, DH = 2, 2048, 2048, 1024, 16, 64
N_CORES = 8
HPC = 4            # heads per core
GROUPS = N_CORES // B  # 4 head-groups
IB = 512           # i-block width (softmax rows per block)
JT = 128           # j-tile height
N_IB = QL // IB
N_JT = KL // JT
KT = D // 128      # contraction tiles for projections


def classify_tiles(attn_mask, key_padding_mask):
    """Per i-block list of j-tiles that have at least one valid entry for at
    least one batch (uniform across cores; fully-masked tiles are skipped)."""
    m = np.asarray(attn_mask, dtype=bool)
    kp = np.asarray(key_padding_mask, dtype=bool)
    kp_any = kp.any(axis=0)  # [KL] valid for some batch
    classes = []
    for t in range(N_IB):
        mi = m[t * IB : (t + 1) * IB]
        row = []
        for jt in range(N_JT):
            v = mi[:, jt * JT : (jt + 1) * JT] & kp_any[jt * JT : (jt + 1) * JT][None, :]
            if v.any():
                row.append(jt)
        classes.append(row)
    return classes


def build_nc(classes, repeats=1, skip=()):
    skip = set(skip)
    """Build the SPMD Bass program.

    skip flags (ablation):
      dverecip-- vector.reciprocal on [1,512] instead of ACT exp(-ln l)
      woact   -- Wo PSUM evac on ACT instead of DVE
    """
    # rpb units are [128, 1024] (headA|headB) per (t, m, jt), DMA'd two
    # j-tiles at a time as [128, 2048]
    n_units = sum(2 * len(row) for row in classes)
    n_pair = n_units // 2

    nc = bacc.Bacc("TRN2", target_bir_lowering=False, debug=False)
    qT = nc.dram_tensor("qT", [D, QL], dt.bfloat16, kind="ExternalInput")
    kTd = nc.dram_tensor("kT", [D, KL], dt.bfloat16, kind="ExternalInput")
    vTd = nc.dram_tensor("vT", [D, KL], dt.bfloat16, kind="ExternalInput")
    wqT = nc.dram_tensor("wqT", [D, HPC * DH], dt.bfloat16, kind="ExternalInput")
    wkT = nc.dram_tensor("wkT", [D, HPC * DH], dt.bfloat16, kind="ExternalInput")
    wvT = nc.dram_tensor("wvT", [D, HPC * DH], dt.bfloat16, kind="ExternalInput")
    woT = nc.dram_tensor("woT", [HPC * DH, D], dt.bfloat16, kind="ExternalInput")
    rpbM = nc.dram_tensor("rpbM", [max(n_pair, 1), JT, 4 * IB],
                          dt.bfloat16, kind="ExternalInput")
    outP = nc.dram_tensor("outP", [D, QL], dt.bfloat16, kind="ExternalOutput")

    Exp = mybir.ActivationFunctionType.Exp
    Ln = mybir.ActivationFunctionType.Ln
    Copy = mybir.ActivationFunctionType.Copy

    with tile.TileContext(nc) as tc:
        with (
            tc.tile_pool(name="wp", bufs=1) as wp,
            tc.tile_pool(name="persist", bufs=1) as pers,
            tc.tile_pool(name="xq", bufs=6) as xq,
            tc.tile_pool(name="ptp", bufs=3) as ptp,
            tc.tile_pool(name="rpbp", bufs=6) as rpbp,
            tc.tile_pool(name="smallp", bufs=2) as smallp,
            tc.tile_pool(name="rbp", bufs=3) as rbp,
            tc.tile_pool(name="osb", bufs=3) as osbp,
            tc.tile_pool(name="psA", bufs=2, space="PSUM") as psA,
            tc.tile_pool(name="psS", bufs=2, space="PSUM") as psS,
            tc.tile_pool(name="psG", bufs=2, space="PSUM") as psG,
        ):

            def body():
                wq_t = wp.tile([128, KT, 256], dt.bfloat16, tag="wq")
                wk_t = wp.tile([128, KT, 256], dt.bfloat16, tag="wk")
                wv_t = wp.tile([128, KT, 256], dt.bfloat16, tag="wv")
                wo_t = wp.tile([128, 2, 1024], dt.bfloat16, tag="wo")

                # chunked persistent activation tiles (fine-grained deps so
                # early attention blocks can start before projections finish)
                qh = [[pers.tile([128, 512], dt.bfloat16, name=f"qh{m}_{c}", tag=f"qh{m}_{c}")
                       for c in range(QL // 512)] for m in range(2)]
                kh = [[pers.tile([128, 512], dt.bfloat16, name=f"kh{m}_{c}", tag=f"kh{m}_{c}")
                       for c in range(KL // 512)] for m in range(2)]
                vh = [pers.tile([128, HPC, 68], dt.bfloat16, name=f"vh{t}", tag=f"vh{t}")
                      for t in range(N_JT)]
                # unnormalized out^T and normalized (bf16) out^T per i-block
                otu = [pers.tile([128, 2, 512], dt.bfloat16, name=f"otu{t}", tag=f"otu{t}")
                       for t in range(N_IB)]
                ot = [pers.tile([128, 2, 512], dt.bfloat16, name=f"ot{t}", tag=f"ot{t}")
                      for t in range(N_IB)]
                # vh ones columns: memset upfront, off any critical path
                for t in range(N_JT):
                    nc.gpsimd.memset(vh[t][:, :, 64:65], 1.0)

                rpb_pre = {}

                def rpb_tile():
                    return rpbp.tile([JT, 2048], dt.bfloat16, tag="rpb", name="rt")

                def prefetch_rpb():
                    for i in range(min(len(classes[0]), 4)):
                        rt = rpb_tile()
                        nc.sync.dma_start(out=rt[:], in_=rpbM[i])
                        rpb_pre[i] = rt

                pending = []  # projection units: popped between j-tiles, and
                              # force-drained at block start
                lazy_rdy = []   # deferred norm/Wo units, poppable now
                lazy_new = []   # emitted this pair-loop; promoted at next pair

                xts = {}  # (kind, c) -> in-flight xt tile

                def dma_trio(c):
                    for src, kind in ((qT, "q"), (kTd, "k"), (vTd, "v")):
                        # one DMA per chunk: each issue costs ~630ns of
                        # serial SP time, which dwarfs fine-slicing benefits
                        xt = xq.tile([128, KT, 512], dt.bfloat16, tag="x", name="xt")
                        nc.sync.dma_start(
                            out=xt[:],
                            in_=src.ap()[:, c * 512 : (c + 1) * 512].rearrange(
                                "(k p) t -> p k t", p=128
                            ),
                        )
                        xts[(kind, c)] = xt

                def enqueue_trio(c):
                    for w_t, kind in ((wq_t, "q"), (wk_t, "k"), (wv_t, "v")):
                        enqueue_one(w_t, kind, c)

                def enqueue_one(w_t, kind, c):
                    xt = xts.pop((kind, c))
                    if kind in ("q", "k"):
                        dst = qh if kind == "q" else kh

                        # kt-chain interleaved across both m outputs (two PSUM
                        # banks) so consecutive MMs overlap fill/drain
                        def qk_unit(k0, k1, pps, xt=xt, w_t=w_t, dst=dst, c=c):
                            for kt in range(k0, k1):
                                for m in range(2):
                                    nc.tensor.matmul(
                                        pps[m][:],
                                        w_t[:, kt, m * 128 : (m + 1) * 128],
                                        xt[:, kt, :],
                                        start=(kt == 0),
                                        stop=(kt == KT - 1),
                                    )
                            if k1 == KT:
                                for m in range(2):
                                    nc.vector.tensor_copy(dst[m][c][:], pps[m][:])

                        pps = [psA.tile([128, 512], dt.float32, tag="mm", name=f"pp{m}")
                               for m in range(2)]
                        for k0, k1 in ((0, KT // 2), (KT // 2, KT)):
                            pending.append(lambda k0=k0, k1=k1, pps=pps, f=qk_unit: f(k0, k1, pps))
                    else:

                        # tsub pairs interleaved across two PSUM banks
                        def v_unit(t0, t1, pvs, xt=xt, c=c):
                            for kt in range(KT):
                                for i, ts in enumerate((t0, t1)):
                                    nc.tensor.matmul(
                                        pvs[i][:],
                                        xt[:, kt, ts * 128 : (ts + 1) * 128],
                                        wv_t[:, kt, :],
                                        start=(kt == 0),
                                        stop=(kt == KT - 1),
                                    )
                            for i, ts in enumerate((t0, t1)):
                                t = c * 4 + ts
                                nc.scalar.activation(
                                    vh[t][:, :, 0:64],
                                    pvs[i][:].rearrange("p (h c) -> p h c", h=HPC),
                                    Copy,
                                )

                        for t0, t1 in ((0, 1), (2, 3)):
                            pvs = [psA.tile([128, 256], dt.float32, tag="mm", name=f"pv{i}")
                                   for i in range(2)]
                            pending.append(lambda t0=t0, t1=t1, pvs=pvs, f=v_unit: f(t0, t1, pvs))

                def wo_unit(t, n0):
                    # two n-tiles interleaved across two PSUM banks
                    pws = [psA.tile([128, 512], dt.float32, tag="mm", name=f"pw{i}")
                           for i in range(2)]
                    for m in range(2):
                        for i in range(2):
                            nc.tensor.matmul(
                                pws[i][:],
                                wo_t[:, m, (n0 + i) * 128 : (n0 + i + 1) * 128],
                                ot[t][:, m, :],
                                start=(m == 0),
                                stop=(m == 1),
                            )
                    for i in range(2):
                        n = n0 + i
                        ob = osbp.tile([128, 512], dt.bfloat16, tag="ob")
                        if "woact" in skip:
                            nc.scalar.activation(ob[:], pws[i][:], Copy)
                        else:
                            nc.vector.tensor_copy(ob[:], pws[i][:])
                        if t == N_IB - 1:
                            # tail-critical: halve per-queue transfer latency
                            for hh in range(2):
                                nc.sync.dma_start(
                                    out=outP[n * 128 : (n + 1) * 128,
                                             t * IB + hh * 256 : t * IB + (hh + 1) * 256],
                                    in_=ob[:, hh * 256 : (hh + 1) * 256],
                                )
                        else:
                            nc.sync.dma_start(
                                out=outP[n * 128 : (n + 1) * 128, t * IB : (t + 1) * IB],
                                in_=ob[:],
                            )

                def pop_pending(allow_lazy=True):
                    if pending:
                        pending.pop(0)()
                    elif lazy_rdy and allow_lazy:
                        lazy_rdy.pop(0)()

                # ---- interleaved: attention i-block t runs while chunk t+1 of
                # the projections streams in between its j-tiles (causal: block
                # t only reads k/v chunks <= t) ----
                rpb_i = 0  # DMA'd rpb pair counter
                # block 0's inputs are emitted eagerly, each weight landing
                # just before the x-chunk that needs it; chunk 1's x-DMAs
                # are issued in the preamble too so block 0 (short) can
                # hide chunk 1's projections without stalling on transfers
                nc.sync.dma_start(out=wq_t[:], in_=wqT.ap().rearrange("(k p) c -> p k c", p=128))
                for src, kind in ((qT, "q"),):
                    xt = xq.tile([128, KT, 512], dt.bfloat16, tag="x", name="xt")
                    nc.sync.dma_start(out=xt[:], in_=src.ap()[:, 0:512].rearrange("(k p) t -> p k t", p=128))
                    xts[(kind, 0)] = xt
                enqueue_one(wq_t, "q", 0)
                while pending:
                    pop_pending()
                nc.sync.dma_start(out=wk_t[:], in_=wkT.ap().rearrange("(k p) c -> p k c", p=128))
                for src, kind in ((kTd, "k"), (vTd, "v")):
                    xt = xq.tile([128, KT, 512], dt.bfloat16, tag="x", name="xt")
                    nc.sync.dma_start(out=xt[:], in_=src.ap()[:, 0:512].rearrange("(k p) t -> p k t", p=128))
                    xts[(kind, 0)] = xt
                enqueue_one(wk_t, "k", 0)
                while pending:
                    pop_pending()
                prefetch_rpb()
                nc.sync.dma_start(out=wv_t[:], in_=wvT.ap().rearrange("(k p) c -> p k c", p=128))
                enqueue_one(wv_t, "v", 0)
                while pending:
                    pop_pending()
                nc.sync.dma_start(out=wo_t[:], in_=woT.ap().rearrange("(k p) c -> p k c", p=128))
                dma_trio(1)

                for t in range(N_IB):
                    row = classes[t]
                    n_row = len(row)
                    while pending:  # anything block t needs must be emitted now
                        pop_pending()
                    if t + 1 < N_IB:
                        enqueue_trio(t + 1)
                    if t + 2 < N_IB:
                        dma_trio(t + 2)
                    for m in range(2):
                        # promote deferred units from the previous pair-loop
                        lazy_rdy.extend(lazy_new)
                        lazy_new.clear()
                        # per-head l rows land in row 0 of a [32,512] tile so
                        # the norm chain can 32x32-transpose them across
                        # partitions (rows 1-31 hold garbage, never read)
                        Lt = [smallp.tile([32, 512], dt.float32, tag="L1", name="Lh")
                              for _ in range(2)]
                        augs = [psG.tile([65, 512], dt.float32, tag="aug", name=f"aug{i}")
                                for i in range(2)]
                        pends = []  # software-pipeline: AV(jt) issues after QK/exp(jt+1)

                        def av_pair(jj, jt, PT, m=m, augs=augs):
                            for i in range(2):
                                nc.tensor.matmul(
                                    augs[i][:],
                                    vh[jt][:, 2 * m + i, 0:65],
                                    PT[:, i * 512 : (i + 1) * 512],
                                    start=(jj == 0),
                                    stop=(jj == n_row - 1),
                                )

                        half = None
                        for jj, jt in enumerate(row):
                            if jj % 2 == 0:
                                # rpb pairs: two j-tiles per DMA
                                if rpb_i in rpb_pre:
                                    rt2 = rpb_pre.pop(rpb_i)
                                else:
                                    rt2 = rpb_tile()
                                    nc.sync.dma_start(out=rt2[:], in_=rpbM[rpb_i])
                                rpb_i += 1
                                half = 0
                            else:
                                half = 1
                            S2 = psS.tile([128, 1024], dt.float32, tag="s2")
                            PT = ptp.tile([128, 1024], dt.bfloat16, tag="pt")
                            for i in range(2):
                                hp = 64 * i
                                nc.tensor.matmul(
                                    S2[:, i * 512 : (i + 1) * 512],
                                    kh[m][jt // 4][hp : hp + 64, (jt % 4) * 128 : (jt % 4 + 1) * 128],
                                    qh[m][t][hp : hp + 64, :],
                                    start=True, stop=True,
                                )
                            nc.scalar.activation(PT[:], S2[:], Exp)
                            nc.vector.tensor_mul(
                                PT[:], PT[:], rt2[:, half * 1024 : (half + 1) * 1024]
                            )
                            pends.append((jj, jt, PT))
                            if len(pends) > 2:
                                av_pair(*pends.pop(0))
                            pop_pending()
                        while pends:
                            av_pair(*pends.pop(0))
                        # pair tail: evacuate augs + l rows, split evenly
                        # across ACT/DVE so neither FIFO blocks the next
                        # pair's exp/mul stream for long (gpsimd cannot
                        # read PSUM)
                        for i in range(2):
                            hp = 64 * i
                            if i == 0:
                                nc.scalar.activation(
                                    otu[t][hp : hp + 64, m, :], augs[i][0:64, :], Copy
                                )
                                nc.scalar.activation(Lt[i][0:1, :], augs[i][64:65, :], Copy)
                            else:
                                nc.vector.tensor_copy(
                                    otu[t][hp : hp + 64, m, :], augs[i][0:64, :]
                                )
                                nc.vector.tensor_copy(Lt[i][0:1, :], augs[i][64:65, :])

                        # deferred normalization for this pair (pops later, so
                        # the ACT FIFO isn't blocked behind the AV chain)
                        def norm_unit(t=t, m=m, Lt=Lt):
                            # 1/l without ACT table switches (Ln thrashes the
                            # exp table set, ~2.6us per swap), without the
                            # single-lane [1,512] reciprocal (8 cyc/elem =
                            # 3.3us), and without DMA spreads (16B/partition
                            # packets = ~5us/transfer): DVE 32x32 stream-
                            # transpose puts l across 32 partitions, a strided
                            # FD=16 reciprocal inverts it, transpose back,
                            # then gpsimd partition-broadcast.
                            for i in range(2):
                                hp = 64 * i
                                lT = smallp.tile([32, 512], dt.float32, tag="lT", name="lT")
                                nc.vector.transpose(lT[:], Lt[i][:])
                                rT = smallp.tile([32, 512], dt.float32, tag="rT", name="rT")
                                nc.vector.reciprocal(
                                    rT[:, 0:512:32], lT[:, 0:512:32]
                                )
                                rc = smallp.tile([32, 512], dt.float32, tag="L1r", name="Lr")
                                nc.vector.transpose(rc[:], rT[:])
                                rb = rbp.tile([128, 512], dt.float32, tag="rb", name="rb")
                                nc.gpsimd.partition_broadcast(rb[:], rc[0:1, :])
                                nc.vector.tensor_mul(
                                    ot[t][hp : hp + 64, m, :],
                                    otu[t][hp : hp + 64, m, :],
                                    rb[hp : hp + 64, :],
                                )

                        lazy_new.append(norm_unit)
                    # Wo partials for this i-block (needs both pairs' ot): pop
                    # during block t+1 instead of stalling the PE behind the
                    # normalization chain
                    for n0 in range(0, 8, 2):
                        lazy_new.append(lambda n0=n0, t=t: wo_unit(t, n0))
                while pending or lazy_rdy or lazy_new:
                    lazy_rdy.extend(lazy_new)
                    lazy_new.clear()
                    while pending or lazy_rdy:
                        pop_pending()

            if repeats == 1:
                body()
            else:
                hint = (mybir.EngineType.PE, mybir.EngineType.Activation,
                        mybir.EngineType.DVE, mybir.EngineType.SP,
                        mybir.EngineType.Pool)
                with tc.For_i(0, repeats, 1, hint_engines=hint):
                    body()

    nc.finalize()
    return nc


def make_in_maps(q, k, v, attn_mask, key_padding_mask, rel_pos_bias, Wq, Wk, Wv, Wo, classes, skip=()):
    q = np.asarray(q, np.float32)
    k = np.asarray(k, np.float32)
    v = np.asarray(v, np.float32)
    Wq = np.asarray(Wq, np.float32)
    Wk = np.asarray(Wk, np.float32)
    Wv = np.asarray(Wv, np.float32)
    Wo = np.asarray(Wo, np.float32)
    rpb = np.asarray(rel_pos_bias, np.float32)
    am = np.asarray(attn_mask, bool)
    kp = np.asarray(key_padding_mask, bool)

    scale = np.float32(1.0 / math.sqrt(DH))
    n_units = sum(2 * len(row) for row in classes)
    n_pair = n_units // 2

    in_maps = []
    for core in range(N_CORES):
        b = core // GROUPS
        g = core % GROUPS
        h0 = g * HPC
        r0 = h0 * DH

        qTc = q[b].T.astype(bf16)
        kTc = k[b].T.astype(bf16)
        vTc = v[b].T.astype(bf16)
        wqTc = ((Wq[r0 : r0 + HPC * DH] * scale).T).astype(bf16)
        wkTc = Wk[r0 : r0 + HPC * DH].T.astype(bf16)
        wvTc = Wv[r0 : r0 + HPC * DH].T.astype(bf16)
        woTc = np.ascontiguousarray(Wo[:, r0 : r0 + HPC * DH].T).astype(bf16)

        # bias tiles: exp(rel_pos_bias)^T where valid, exactly 0 where masked.
        # Unit = [128, 1024] (headA|headB) per (t, m, jt); packed two j-tiles
        # per DMA row in device iteration order.
        validT = (am & kp[b][None, :]).T  # [KL, QL]
        rpbm_arr = np.zeros((max(n_pair, 1), JT, 2048), dtype=bf16)
        iu = 0
        for t in range(N_IB):
            ts = slice(t * IB, (t + 1) * IB)
            for m in range(2):
                row = classes[t]
                for jj, jt in enumerate(row):
                    js = slice(jt * JT, (jt + 1) * JT)
                    vT = validT[js, ts]
                    pr, half = divmod(iu, 2)
                    for i in range(2):
                        rT = rpb[h0 + 2 * m + i].T  # [KL, QL] view
                        tilev = np.where(vT, np.exp(rT[js, ts]), 0.0).astype(bf16)
                        off = half * 1024 + i * 512
                        rpbm_arr[pr, :, off : off + 512] = tilev
                    iu += 1
        assert iu == n_units

        in_maps.append(
            {
                "qT": qTc, "kT": kTc, "vT": vTc,
                "wqT": wqTc, "wkT": wkTc, "wvT": wvTc, "woT": woTc,
                "rpbM": rpbm_arr,
            }
        )
    return in_maps


_CACHE = {}


def _get_nc(classes, repeats=1, skip=()):
    key = (tuple(tuple(row) for row in classes), repeats, tuple(sorted(skip)))
    if key not in _CACHE:
        _CACHE[key] = build_nc(classes, repeats, skip)
    return _CACHE[key]


def kernel(q, k, v, attn_mask, key_padding_mask, rel_pos_bias, Wq, Wk, Wv, Wo):
    classes = classify_tiles(attn_mask, key_padding_mask)
    nc = _get_nc(classes)
    in_maps = make_in_maps(
        q, k, v, attn_mask, key_padding_mask, rel_pos_bias, Wq, Wk, Wv, Wo, classes
    )
    res = run_bass_kernel_spmd(nc, in_maps, list(range(N_CORES))).results
    out = np.zeros((B, QL, D), np.float32)
    for core in range(N_CORES):
        out[core // GROUPS] += res[core]["outP"].T
    return out
